# revision 45
# baseline (speedup 1.0000x reference)
"""AdaptiveGridKANLayer on 8 TRN2 NeuronCores.

out[b,o] = sum_i sum_g exp(-((x[b,i]-c_g)/w)^2) * coeffs[o,i,g]
         + sum_i silu(x[b,i]) * base_w[o,i]

B=65536, in=out=128, G=8, centers = linspace(-1,1,8), w = 2/7.

Strategy (data-parallel over batch, weights replicated):
- Host: transpose x to feature-major [128, B], shard columns 8 ways; fold the
  Gaussian factorization constants e^(7g-g^2) into the coeffs.
- Device, per core (u = (x+1)/w): basis_g = e^(-(u-g)^2) = p * s^g * const
  with p = exp(-u^2) (ScalarE Square+Exp), s = exp(7x) (ScalarE Exp).
  VectorE builds the power chain t_g = t_{g-1} * s (bf16 2x-mode);
  TensorE contracts tile-major (g inner) per 512-col accumulation group.
- PSUM: 8 banks = 4 rotating slots of 1024-col f32 tiles. Main tiles
  M0..M7; M0..M3 (cols 0..4095) close at g=7 (early, chain-paced) so their
  slots recycle for M4..M7; M4..M7 are closed late by their silu matmuls
  (silu activations exist only after the one exp->silu table switch).
  M0..M3's silu contribution runs afterwards as single-MM groups P0..P3 in
  recycled slots, drained to a separate partial "outs" that the host adds
  during the unshard.
- ScalarE stream order (enforced): exp acts chunk 0..3 with the early M0..M3
  drain copies placed right after later chunks' acts (inside real pacing
  slack, never blocking the chain feed), one table switch, silu acts
  (gen-2 cols first), then its share of late drain copies.  VectorE: chain
  only, then late copies.  All engine op order is pinned (sync=False deps);
  tensor stays tile-major (g-major provokes an SBUF producer-consumer
  conflict that slows DVE/ACT ~20%).
"""

import numpy as np

BATCH = 65536
GRID = 8
NCORES = 8
BLOC = BATCH // NCORES  # 8192 batch columns per core
FDP = 512  # matmul free dim / accumulation group width
TW = 1024  # psum tile width (2 banks); 2 groups per tile
NTILE = BLOC // TW  # 8 main psum tiles
G1 = 4  # main tiles 0..3 close early at g=7; 4..7 close via silu MM
W = 2.0 / (GRID - 1)

FDE = 2048
CHUNKS = [[256, 256, 1536], [2048], [2048], [2048]]

_NC = None


def _build():
    import concourse.mybir as mybir
    from concourse import bacc
    from concourse.tile import TileContext, add_dep_helper

    AF = mybir.ActivationFunctionType
    bf16 = mybir.dt.bfloat16
    f32 = mybir.dt.float32

    nc = bacc.Bacc("TRN2", num_devices=NCORES)
    # Bias constant for the Square activation. Must be a raw (non-pool)
    # tensor: const_aps captures the AP before pool relocation. vector
    # memset, NOT gpsimd (a single gpsimd op drags a ~6us Q7 library load
    # into the preamble).  No all_engine_barrier — it would hold the x DMA
    # stream behind the memset; a single targeted dep on the first Square
    # act (the only cst reader) replaces it.
    cst = nc.alloc_sbuf_tensor("const-float32-bias-c", [128, 1], f32)
    nc.const_aps.aps[(f32, 1.0 / W)] = cst.ap()
    xt = nc.dram_tensor("xt", [128, BLOC], f32, kind="ExternalInput").ap()
    wt = nc.dram_tensor("wt", [128, 9 * 128], bf16, kind="ExternalInput").ap()
    out = nc.dram_tensor("out", [128, BLOC], bf16, kind="ExternalOutput").ap()
    # silu partials: x-cols [0:4096] at outs[0:4096], x-cols [6144:8192]
    # (M6/M7, which close at g7 so the tail never waits on their silu) at
    # outs[4096:6144].
    outs = nc.dram_tensor(
        "outs", [128, 6 * TW], bf16, kind="ExternalOutput"
    ).ap()

    with TileContext(nc) as tc:
        with (
            tc.tile_pool(name="const", bufs=1) as cpool,
            tc.tile_pool(name="work", bufs=3) as wpool,
            tc.tile_pool(name="obuf", bufs=6) as opool,
            tc.tile_pool(name="psum", bufs=4, space="PSUM") as ppool,
        ):
            # cst memset inside the context so the targeted dep edge from
            # the first Square act resolves in the tile scheduler.
            cst_op = nc.vector.memset(cst.ap(), 1.0 / W)

            # Exp table load during preamble (no DMA deps).
            warm_act = cpool.tile([128, 1], f32, name="warm_act")
            nc.vector.memset(warm_act[:], 0.0)
            nc.scalar.activation(warm_act[:], warm_act[:], AF.Exp, scale=1.0)

            # PE HAM clock warm: memset-fed matmuls (no DMA deps).
            wm_s = cpool.tile([128, 128], bf16, name="wm_s")
            wm_m = cpool.tile([128, 256], bf16, name="wm_m")
            nc.vector.memset(wm_s[:], 0.25)
            nc.vector.memset(wm_m[:], 0.25)
            warm_ps = ppool.tile([128, 256], f32, name="warm_ps", tag="psum")
            for _ in range(10):
                nc.tensor.matmul(
                    warm_ps[:], wm_s[:], wm_m[:], start=True, stop=True
                )

            # x stream alternating across BOTH HWDGE rings (sync + scalar):
            # a single ring serializes the pieces and paces the whole left
            # half of the pipeline.  Weights ride the scalar ring early.
            w_sb = cpool.tile([128, 9, 128], bf16, name="w_sb")
            x_all = cpool.tile([128, BLOC], f32, name="x_all")
            lo = 0
            for i, wd in enumerate([p for ch in CHUNKS for p in ch]):
                eng = nc.sync if i % 2 == 0 else nc.scalar
                eng.dma_start(x_all[:, lo : lo + wd], xt[:, lo : lo + wd])
                lo += wd
                if i == 2:
                    nc.scalar.dma_start(
                        w_sb[:], wt.rearrange("p (g o) -> p g o", g=9)
                    )

            psums = [None] * NTILE
            prev_op = {"s": None, "v": None}

            def order(eng, op):
                if prev_op[eng] is not None:
                    add_dep_helper(op.ins, prev_op[eng].ins, False, "order")
                prev_op[eng] = op
                return op

            def emit_copy(m, engine, ordered=True):
                ob = opool.tile([128, TW], bf16, tag="ob", name=f"ob_{m}")
                if engine == "s":
                    op = nc.scalar.copy(ob[:], psums[m][:])
                else:
                    op = nc.vector.tensor_copy(ob[:], psums[m][:])
                if ordered:
                    order(engine, op)
                nc.scalar.dma_start(out[:, m * TW : (m + 1) * TW], ob[:])

            # ---- exp phase: scalar acts, vector chain, tensor tile-major.
            # Gauss matmuls are emitted per completed 512-col block (pieces
            # can be narrower than a block).  Chunk 3's gauss matmuls are
            # deferred so the tensor tail can interleave silu/partial work.
            tgs = [None] * len(CHUNKS)

            def emit_gauss(m, poff, tg, goff, last_stop):
                for g in range(GRID):
                    nc.tensor.matmul(
                        psums[m][:, poff : poff + FDP],
                        w_sb[:, g, :],
                        tg[g][:, goff : goff + FDP],
                        start=(g == 0),
                        stop=(g == GRID - 1 and last_stop),
                    )

            lo = 0
            for c, pieces in enumerate(CHUNKS):
                s = wpool.tile([128, FDE], bf16, tag="s", name=f"s_{c}")
                q = wpool.tile([128, FDE], f32, tag="q", name=f"q_{c}")
                tg = [
                    wpool.tile([128, FDE], bf16, tag=f"t{g}", name=f"t{g}_{c}")
                    for g in range(GRID)
                ]
                tgs[c] = tg
                psums[2 * c] = ppool.tile(
                    [128, TW], f32, tag="psum", name=f"psum_{2 * c}"
                )
                psums[2 * c + 1] = ppool.tile(
                    [128, TW], f32, tag="psum", name=f"psum_{2 * c + 1}"
                )
                off = 0
                mm_lo = 0
                for wd in pieces:
                    hs = slice(off, off + wd)
                    xc = x_all[:, lo + off : lo + off + wd]
                    order(
                        "s",
                        nc.scalar.activation(s[:, hs], xc, AF.Exp, scale=2.0 / W),
                    )
                    sq_op = order(
                        "s",
                        nc.scalar.activation(
                            q[:, hs], xc, AF.Square, bias=1.0 / W, scale=1.0 / W
                        ),
                    )
                    if c == 0 and off == 0:
                        add_dep_helper(sq_op.ins, cst_op.ins, True, "cst ready")
                    order(
                        "s",
                        nc.scalar.activation(
                            tg[0][:, hs], q[:, hs], AF.Exp, scale=-1.0
                        ),
                    )
                    for g in range(1, GRID):
                        order(
                            "v",
                            nc.vector.tensor_mul(
                                tg[g][:, hs], tg[g - 1][:, hs], s[:, hs]
                            ),
                        )
                    off += wd
                    if c < 3:
                        while mm_lo + FDP <= off:
                            m = (lo + mm_lo) // TW
                            emit_gauss(
                                m, (lo + mm_lo) % TW, tg, mm_lo, m < G1
                            )
                            mm_lo += FDP
                lo += FDE
                # early drains of the g7-closed tiles, placed in the scalar
                # stream late enough to be data-ready (no queue stall) but
                # before their psum slot is needed again (M_{k+4}'s birth).
                if c == 2:
                    emit_copy(0, "s")
                    emit_copy(1, "s")
                if c == 3:
                    emit_copy(2, "s")
                    emit_copy(3, "s")

            # ---- silu phase: one table switch.  Act order: M4/M5 cols
            # (closes recycle slots 1/2 for P0/P1), then P0/P1 cols, then
            # M6/M7 cols, then P2/P3 cols.  The M4/M5 drain copies sit
            # between act0 and act1 on the scalar stream.
            silu_sb = cpool.tile([128, BLOC], bf16, name="silu_sb")

            def silu_act(k0):
                ks = slice(k0, k0 + 2048)
                order(
                    "s",
                    nc.scalar.activation(silu_sb[:, ks], x_all[:, ks], AF.Silu),
                )

            def silu_mm(ps, poff, kl, start):
                nc.tensor.matmul(
                    ps[:, poff : poff + FDP],
                    w_sb[:, 8, :],
                    silu_sb[:, kl : kl + FDP],
                    start=start,
                    stop=True,
                )

            ndma = [0]

            def late_copy(ps, engine, dram, col):
                ob = opool.tile([128, TW], bf16, tag="ob", name=f"lob_{col}")
                if engine == "s":
                    order("s", nc.scalar.copy(ob[:], ps[:]))
                else:
                    order("v", nc.vector.tensor_copy(ob[:], ps[:]))
                deng = nc.sync if ndma[0] % 2 == 0 else nc.scalar
                ndma[0] += 1
                deng.dma_start(dram[:, col : col + TW], ob[:])

            # NOTE: emission order IS dependency order for psum readers — a
            # copy emitted before the silu MM would read a gauss-only sum
            # (the later MM becomes a dead WAR write).  Close M4/M5 first.
            # All silu acts run back-to-back (the M4..M7 closes are gated by
            # the vector chain's tail anyway; copies interleaved here would
            # stall the act stream for ~3us waiting on t7 of chunk 2).
            silu_act(4096)
            for m in (4, 5):
                for sub in range(2):
                    silu_mm(psums[m], sub * FDP, m * TW + sub * FDP, False)
            silu_act(6144)
            silu_act(0)
            silu_act(2048)
            late_copy(psums[4], "s", out, 4 * TW)
            late_copy(psums[5], "s", out, 5 * TW)

            tg3 = tgs[3]
            for mi, m in enumerate((6, 7)):
                for sub in range(2):
                    goff = (mi * TW) + sub * FDP
                    for g in range(GRID - 1):
                        nc.tensor.matmul(
                            psums[m][:, sub * FDP : sub * FDP + FDP],
                            w_sb[:, g, :],
                            tg3[g][:, goff : goff + FDP],
                            start=(g == 0),
                            stop=False,
                        )

            # chain-gated: g7s CLOSE M6/M7 (stop=True — their silu goes to
            # partials so the tail never serializes behind the table phase).
            for mi, m in enumerate((6, 7)):
                for sub in range(2):
                    goff = (mi * TW) + sub * FDP
                    nc.tensor.matmul(
                        psums[m][:, sub * FDP : sub * FDP + FDP],
                        w_sb[:, 7, :],
                        tg3[7][:, goff : goff + FDP],
                        start=False,
                        stop=True,
                    )

            # partial tiles in slot-recycle order: P4/P5 (x-cols 6144..8191)
            # first (slots from the M4/M5 drains), then P0..P3.
            pcols = [6144, 7168, 0, 1024, 2048, 3072]
            pss = []
            for i, pc in enumerate(pcols):
                ps = ppool.tile([128, TW], f32, tag="psum", name=f"psilu_{i}")
                pss.append(ps)
                for sub in range(2):
                    silu_mm(ps, sub * FDP, pc + sub * FDP, True)

            # late drains, readiness-ordered per engine; DMAs alternate the
            # idle rings.  outs layout: P0..P3 at [0:4096], P4/P5 at
            # [4096:6144].
            late_copy(psums[6], "v", out, 6 * TW)
            late_copy(psums[7], "s", out, 7 * TW)
            late_copy(pss[0], "v", outs, 4 * TW)  # P4 (x-cols 6144)
            late_copy(pss[1], "s", outs, 5 * TW)  # P5 (x-cols 7168)
            late_copy(pss[2], "v", outs, 0)  # P0
            late_copy(pss[3], "s", outs, 1 * TW)  # P1
            late_copy(pss[4], "v", outs, 2 * TW)  # P2

            ps3 = pss[5]
            for h in range(2):
                obh = opool.tile([128, FDP], bf16, tag="obh", bufs=2, name=f"obh{h}")
                order("s", nc.scalar.copy(obh[:], ps3[:, h * FDP : (h + 1) * FDP]))
                deng = nc.sync if h == 0 else nc.scalar
                deng.dma_start(
                    outs[:, 3 * TW + h * FDP : 3 * TW + (h + 1) * FDP], obh[:]
                )

    nc.compile()
    return nc


def _prep_weights(coeffs, base_w):
    import ml_dtypes

    g = np.arange(GRID, dtype=np.float64)
    K = np.exp(7.0 * g - g * g)  # t_g = basis_g * e^(g^2-7g) -> fold inverse
    blocks = [
        (coeffs[:, :, gi].astype(np.float64) * K[gi]).T for gi in range(GRID)
    ]  # [in, out] each
    blocks.append(base_w.astype(np.float64).T)
    wtm = np.concatenate(blocks, axis=1)  # [128, 9*128]
    return np.ascontiguousarray(wtm.astype(ml_dtypes.bfloat16))


def _gather(results):
    """Merge per-core outputs: out + silu partials (cols 0:4096, 6144:8192)."""
    cols = []
    for c in range(NCORES):
        full = results[c]["out"].astype(np.float32)  # [128, BLOC]
        part = results[c]["outs"].astype(np.float32)  # [128, 6*TW]
        full[:, : 4 * TW] += part[:, : 4 * TW]
        full[:, 6 * TW :] += part[:, 4 * TW :]
        cols.append(full)
    return np.ascontiguousarray(np.concatenate(cols, axis=1).T)


def kernel(x, coeffs, base_w, centers):
    from concourse.bass_utils import run_bass_kernel_spmd

    global _NC
    if _NC is None:
        _NC = _build()

    wtm = _prep_weights(coeffs, base_w)
    xT = np.ascontiguousarray(np.asarray(x, dtype=np.float32).T)  # [128, B]
    in_maps = [
        {
            "xt": np.ascontiguousarray(xT[:, c * BLOC : (c + 1) * BLOC]),
            "wt": wtm,
        }
        for c in range(NCORES)
    ]
    res = run_bass_kernel_spmd(_NC, in_maps, list(range(NCORES)))
    return _gather(res.results)
